# revision 10
# baseline (speedup 1.0000x reference)
"""Causal self-attention (RoPE, 16 heads) Trainium2 Bass kernel.

Problem: B=8, S=1024, D=1024, H=16, HS=64, fp32, causal + all-ones padding mask.

Strategy: data-parallel over batch — one batch element per NeuronCore (8 cores).
All matmul data is bf16 (fp32 PSUM accumulation); host does free layout prep
(transpose of x, weight column permutation, coefficient tables).

  x^T   [D, S]   bf16, transposed on host, DMA'd directly.
  Q^T,K^T [D,S]  = W^T @ x^T (lhsT = W chunk, rhs = x^T chunk). RoPE pairs are
                 arranged within 32-partition quadrants (16 x1 | 16 x2) by a
                 host-side W column permutation, so the rotate-half partner is
                 a single DVE stream_shuffle. Rope: ACT cast psum->bf16, DVE
                 shuffle + 2 mul + add with bf16 coefficient tables. Q scaled
                 by 1/sqrt(hs) via its tables.
  V     [S, D]   = x @ W_v (lhsT = x^T chunk, rhs = W_v), stored per-head with
                 an appended ones-column so att@v also yields softmax sums.
  S^T   [k, q]   = K^T-chunks @ Q^T per head, causal blocks only, and only the
                 causally legal column span [qsub:512] of each block.
  att^T          = exp(S^T) on ACT (bf16 out), diagonal 128x128 sub-block
                 masked by a host 0/1 triangle tile (DVE bf16 mul).
  y^T   [D, S]   accumulated per head: lhsT = [v | 1] chunk, rhs = att^T span;
                 row 64 gives softmax sums. Normalize: DVE reciprocal of the
                 sum row, gpsimd partition_broadcast, one DVE mul that also
                 evicts psum -> yt bf16.
  out^T [D, S]   = W_p^T @ y^T (lhsT = wp chunk, rhs = y^T), fp32, transposed
                 back on host.

Emission order interleaves QK projection (fc) with attention (ft = fc-1) so
ACT exp work overlaps projection PE work.
"""

import os

# The Bass kernel executes through the axon PJRT backend and needs the
# NeuronCores visible; a JAX_PLATFORMS=cpu pin (used for jax reference
# computation) would hide them.
if "axon" not in os.environ.get("JAX_PLATFORMS", "axon"):
    os.environ.pop("JAX_PLATFORMS", None)

import numpy as np
import ml_dtypes
from contextlib import ExitStack

import concourse.bass as bass
import concourse.mybir as mybir
import concourse.tile as tile
from concourse import bacc
from concourse.bass_utils import run_bass_kernel_spmd

B, S, D, H, HS = 8, 1024, 1024, 16, 64
P = 128
NCORES = 8
F32 = mybir.dt.float32
BF16 = mybir.dt.bfloat16
EXP = mybir.ActivationFunctionType.Exp
BFNP = ml_dtypes.bfloat16

# Swap the two 16-row halves of each 32-partition quadrant (rotate-half
# partner exchange for the quadrant-pair RoPE layout).
SHUF_MASK = [(i + 16) % 32 for i in range(32)]

_CACHE = {}


def _build_nc():
    nc = bacc.Bacc(
        "TRN2", target_bir_lowering=False, debug=False, num_devices=NCORES)
    xT_d = nc.dram_tensor("xT", [D, S], BF16, kind="ExternalInput")
    wq_d = nc.dram_tensor("wq", [D, D], BF16, kind="ExternalInput")
    wk_d = nc.dram_tensor("wk", [D, D], BF16, kind="ExternalInput")
    wv_d = nc.dram_tensor("wv", [D, D], BF16, kind="ExternalInput")
    wp_d = nc.dram_tensor("wp", [D, D], BF16, kind="ExternalInput")
    c1q_d = nc.dram_tensor("c1q", [P, S], BF16, kind="ExternalInput")
    c2q_d = nc.dram_tensor("c2q", [P, S], BF16, kind="ExternalInput")
    c1k_d = nc.dram_tensor("c1k", [P, S], BF16, kind="ExternalInput")
    c2k_d = nc.dram_tensor("c2k", [P, S], BF16, kind="ExternalInput")
    mask_d = nc.dram_tensor("mask", [P, P], BF16, kind="ExternalInput")
    onesH_d = nc.dram_tensor("onesH", [P, H], BF16, kind="ExternalInput")
    outT_d = nc.dram_tensor("outT", [D, S], F32, kind="ExternalOutput")

    def mm(out, lhsT, rhs, start, stop):
        nc.tensor.matmul(out, lhsT, rhs, start=start, stop=stop)

    with tile.TileContext(nc) as tc, ExitStack() as ctx:
        persist = ctx.enter_context(tc.tile_pool(name="persist", bufs=1))
        xt = [persist.tile([P, S], BF16, name=f"xt{i}", tag=f"xt{i}") for i in range(8)]
        qt = [persist.tile([P, S], BF16, name=f"qt{i}", tag=f"qt{i}") for i in range(8)]
        kt = [persist.tile([P, S], BF16, name=f"kt{i}", tag=f"kt{i}") for i in range(8)]
        vt = [persist.tile([P, H, HS + 1], BF16, name=f"vt{i}", tag=f"vt{i}")
              for i in range(8)]
        yt = [persist.tile([P, S], BF16, name=f"yt{i}", tag=f"yt{i}") for i in range(8)]
        wqt = [persist.tile([P, S], BF16, name=f"wqt{i}", tag=f"wqt{i}") for i in range(8)]
        wkt = [persist.tile([P, S], BF16, name=f"wkt{i}", tag=f"wkt{i}") for i in range(8)]
        wpt = [persist.tile([P, S], BF16, name=f"wpt{i}", tag=f"wpt{i}") for i in range(8)]
        c1q = persist.tile([P, S], BF16, name="c1q_t", tag="c1q_t")
        c2q = persist.tile([P, S], BF16, name="c2q_t", tag="c2q_t")
        c1k = persist.tile([P, S], BF16, name="c1k_t", tag="c1k_t")
        c2k = persist.tile([P, S], BF16, name="c2k_t", tag="c2k_t")
        maskt = persist.tile([P, P], BF16, name="maskt", tag="maskt")
        onesH = persist.tile([P, H], BF16, name="onesH_t", tag="onesH_t")

        # PSUM pools: 2 + 4 + 2 = 8 banks.
        pbc = ctx.enter_context(tc.tile_pool(name="pbc", bufs=2, space="PSUM"))
        pss = ctx.enter_context(tc.tile_pool(name="pss", bufs=4, space="PSUM"))
        psy = ctx.enter_context(tc.tile_pool(name="psy", bufs=2, space="PSUM"))
        ropep = ctx.enter_context(tc.tile_pool(name="ropep", bufs=6))
        smallp = ctx.enter_context(tc.tile_pool(name="smallp", bufs=6))
        outp = ctx.enter_context(tc.tile_pool(name="outp", bufs=3))

        # wvt is only needed for phase C — scope it so its 16KB/partition is
        # released to attp afterwards.
        with tc.tile_pool(name="wvp", bufs=1) as wvp:
            wvt = [wvp.tile([P, S], BF16, name=f"wvt{i}", tag=f"wvt{i}")
                   for i in range(8)]
            # DMA order: x and wv first (V phase starts first), interleaved so
            # the dc-ordered V accumulation can start ASAP; then rope tables
            # and wq/wk (QK phase), wp last (output projection).
            for i in range(8):
                nc.sync.dma_start(xt[i][:], xT_d[i * P:(i + 1) * P, :])
                nc.sync.dma_start(wvt[i][:], wv_d[i * P:(i + 1) * P, :])
            for t, d_ in ((c1q, c1q_d), (c2q, c2q_d), (c1k, c1k_d),
                          (c2k, c2k_d), (maskt, mask_d), (onesH, onesH_d)):
                nc.sync.dma_start(t[:], d_[:])
            for i in range(8):
                nc.sync.dma_start(wqt[i][:], wq_d[i * P:(i + 1) * P, :])
            for i in range(8):
                nc.sync.dma_start(wkt[i][:], wk_d[i * P:(i + 1) * P, :])
            for i in range(8):
                nc.sync.dma_start(wpt[i][:], wp_d[i * P:(i + 1) * P, :])

            # ---------------- Phase C: V = x @ W_v ----------------
            for sc in range(8):
                for f2 in range(2):
                    ps = pbc.tile([P, 512], F32, name="vps", tag="pbc")
                    for dc in range(8):
                        mm(ps[:], xt[dc][:, sc * P:(sc + 1) * P],
                           wvt[dc][:, f2 * 512:(f2 + 1) * 512], dc == 0, dc == 7)
                    nc.scalar.copy(
                        vt[sc][:, f2 * 8:(f2 + 1) * 8, 0:HS],
                        ps[:].rearrange("p (h e) -> p h e", e=HS))
                nc.gpsimd.tensor_copy(vt[sc][:, :, HS], onesH[:])

        attp = ctx.enter_context(tc.tile_pool(name="attp", bufs=25))

        # ---------------- Phase B: Q^T/K^T + RoPE (per fc) ----------------
        def emit_B(fc):
            for wt, dstt, c1, c2 in ((wqt, qt, c1q, c2q), (wkt, kt, c1k, c2k)):
                for s2 in range(2):
                    ps = pbc.tile([P, 512], F32, name="qkps", tag="pbc")
                    for dc in range(8):
                        mm(ps[:], wt[dc][:, fc * P:(fc + 1) * P],
                           xt[dc][:, s2 * 512:(s2 + 1) * 512], dc == 0, dc == 7)
                    s0 = s2 * 512
                    qraw = ropep.tile([P, 512], BF16, name="qraw", tag="rope")
                    nc.scalar.copy(qraw[:], ps[:])
                    swp = ropep.tile([P, 512], BF16, name="swp", tag="rope")
                    nc.vector.stream_shuffle(swp[:], qraw[:], SHUF_MASK)
                    dst = dstt[fc][:, s0:s0 + 512]
                    nc.vector.tensor_mul(dst, qraw[:], c1[:, s0:s0 + 512])
                    t = ropep.tile([P, 512], BF16, name="ropet", tag="rope")
                    nc.vector.tensor_mul(t[:], swp[:], c2[:, s0:s0 + 512])
                    nc.vector.tensor_add(dst, dst, t[:])

        # ---------------- Phase D: scores + att@V + normalize (per ft) -----
        def emit_D(ft):
            # All score blocks (both qc) first, then all att@V, so the PE has
            # a deep runway of score matmuls while ACT exps chase; att@V never
            # waits on an exp that was issued just ahead of it.
            atts = {}
            for qc in range(2):
                kmax = 4 if qc == 0 else 8
                for kc in range(kmax):
                    dq = kc * P - qc * 512
                    qsub = max(0, dq)
                    for hb in (0, 64):
                        pst = pss.tile([P, 512], F32, name="pss", tag="pss")
                        mm(pst[:, qsub:],
                           kt[ft][hb:hb + 64, kc * P:(kc + 1) * P],
                           qt[ft][hb:hb + 64, qc * 512 + qsub:(qc + 1) * 512],
                           True, True)
                        att = attp.tile([P, 512], BF16, name="att", tag="att")
                        nc.scalar.activation(att[:, qsub:], pst[:, qsub:], EXP)
                        if 0 <= dq < 512:
                            nc.vector.tensor_mul(
                                att[:, dq:dq + P], att[:, dq:dq + P], maskt[:])
                        atts[(qc, kc, hb)] = (att, qsub)
            for qc in range(2):
                kmax = 4 if qc == 0 else 8
                for hb in (0, 64):
                    h = 2 * ft + hb // 64
                    pyt = psy.tile([HS + 1, 512], F32, name="psy", tag="psy")
                    for kc in range(kmax):
                        att, qsub = atts[(qc, kc, hb)]
                        mm(pyt[:, qsub:], vt[kc][:, h, :], att[:, qsub:],
                           kc == 0, kc == kmax - 1)
                    # reciprocal_approx_fast must not read PSUM (bitwise seed
                    # reads garbage) — stage the sum row through SBUF.
                    srow = smallp.tile([1, 512], F32, name="srow", tag="rsb")
                    nc.vector.tensor_copy(srow[:], pyt[HS:HS + 1, :])
                    r_sb = smallp.tile([1, 512], F32, name="rsb", tag="rsb")
                    nc.vector.reciprocal_approx_fast(
                        out=r_sb[:], in_=srow[:])
                    rb = smallp.tile([64, 512], F32, name="rb", tag="rb")
                    nc.gpsimd.partition_broadcast(rb[:], r_sb[:])
                    nc.vector.tensor_mul(
                        yt[ft][hb:hb + 64, qc * 512:(qc + 1) * 512],
                        pyt[0:HS, :], rb[:])

        # Interleave: B(0), B(1), D(0), B(2), D(1), ..., B(7), D(6), D(7)
        emit_B(0)
        emit_B(1)
        for ft in range(7):
            emit_D(ft)
            if ft + 2 < 8:
                emit_B(ft + 2)
        emit_D(7)

        # ---------------- Phase E: out^T = W_p^T @ y^T ----------------
        for n8 in range(8):
            for qs in range(2):
                psp = pbc.tile([P, 512], F32, name="psp", tag="pbc")
                for dc in range(8):
                    mm(psp[:], wpt[dc][:, n8 * P:(n8 + 1) * P],
                       yt[dc][:, qs * 512:(qs + 1) * 512], dc == 0, dc == 7)
                ot = outp.tile([P, 512], F32, name="ot", tag="ot")
                # Alternate evict engines: ACT is still chewing D7 exps when
                # E starts, so give half the evictions to DVE.
                if (n8 + qs) % 2 == 0:
                    nc.vector.tensor_copy(ot[:], psp[:])
                else:
                    nc.scalar.copy(ot[:], psp[:])
                nc.sync.dma_start(
                    outT_d[n8 * P:(n8 + 1) * P, qs * 512:(qs + 1) * 512], ot[:])
    nc.compile()
    return nc


def _prep(inputs):
    w_qkv = np.asarray(inputs["w_qkv"], np.float32)
    w_proj = np.asarray(inputs["w_proj"], np.float32)
    wq, wk, wv = w_qkv[:, 0:D], w_qkv[:, D:2 * D], w_qkv[:, 2 * D:3 * D]

    # Quadrant-pair RoPE layout. Within each head's 64 columns, new column i:
    #   qd = i//32 (quadrant), r = i%32, comp = r//16 (x1/x2), fl = r%16
    #   frequency f = qd*16 + fl ; original column = 2f + comp
    i = np.arange(64)
    qd, r = i // 32, i % 32
    comp, fl = r // 16, r % 16
    f = qd * 16 + fl
    base = np.repeat(np.arange(H) * 64, 64)
    perm = base + np.tile(2 * f + comp, H)
    wq, wk = wq[:, perm], wk[:, perm]

    # Coefficient tables [128, S]: rows repeat the 64-row head pattern twice.
    theta = 10000.0
    inv_freq = 1.0 / (theta ** (np.arange(0, HS, 2, dtype=np.float64) / HS))
    pos = np.arange(S, dtype=np.float64)
    ang = np.outer(inv_freq[f], pos)  # [64, S]
    sign = np.where(comp == 1, 1.0, -1.0)[:, None]
    c1_64 = np.cos(ang)
    c2_64 = sign * np.sin(ang)
    c1 = np.concatenate([c1_64, c1_64], 0).astype(np.float32)
    c2 = np.concatenate([c2_64, c2_64], 0).astype(np.float32)
    scale = np.float32(1.0 / np.sqrt(HS))

    mask = np.triu(np.ones((P, P), np.float32))  # [k, q]: allow q >= k
    common = {
        "wq": np.ascontiguousarray(wq).astype(BFNP),
        "wk": np.ascontiguousarray(wk).astype(BFNP),
        "wv": np.ascontiguousarray(wv).astype(BFNP),
        "wp": np.ascontiguousarray(w_proj).astype(BFNP),
        "c1q": (c1 * scale).astype(BFNP), "c2q": (c2 * scale).astype(BFNP),
        "c1k": c1.astype(BFNP), "c2k": c2.astype(BFNP),
        "mask": mask.astype(BFNP),
        "onesH": np.ones((P, H), BFNP),
    }
    return common


LAST_RESULT = None


def kernel(**inputs):
    global LAST_RESULT
    if "nc" not in _CACHE:
        _CACHE["nc"] = _build_nc()
    nc = _CACHE["nc"]
    common = _prep(inputs)
    x = np.asarray(inputs["x"], np.float32)
    in_maps = [
        dict(common, xT=np.ascontiguousarray(x[b].T).astype(BFNP))
        for b in range(B)
    ]
    res = run_bass_kernel_spmd(nc, in_maps, list(range(NCORES)))
    LAST_RESULT = res
    out = np.stack(
        [np.asarray(res.results[i]["outT"]).T for i in range(B)], 0)
    return np.ascontiguousarray(out).astype(np.float32)


# revision 11
# speedup vs baseline: 1.1004x; 1.1004x over previous
"""Causal self-attention (RoPE, 16 heads) Trainium2 Bass kernel.

Problem: B=8, S=1024, D=1024, H=16, HS=64, fp32, causal + all-ones padding mask.

Strategy: data-parallel over batch — one batch element per NeuronCore (8 cores).
All matmul data is bf16 (fp32 PSUM accumulation); host does free layout prep
(transpose of x, weight column permutation, coefficient tables).

  x^T   [D, S]   bf16, transposed on host, DMA'd directly.
  Q^T,K^T [D,S]  = W^T @ x^T (lhsT = W chunk, rhs = x^T chunk). RoPE pairs are
                 arranged within 32-partition quadrants (16 x1 | 16 x2) by a
                 host-side W column permutation, so the rotate-half partner is
                 a single DVE stream_shuffle. Rope: ACT cast psum->bf16, DVE
                 shuffle + 2 mul + add with bf16 coefficient tables. Q scaled
                 by 1/sqrt(hs) via its tables.
  V     [S, D]   = x @ W_v (lhsT = x^T chunk, rhs = W_v), stored per-head with
                 an appended ones-column so att@v also yields softmax sums.
  S^T   [k, q]   = K^T-chunks @ Q^T per head, causal blocks only, and only the
                 causally legal column span [qsub:512] of each block.
  att^T          = exp(S^T) on ACT (bf16 out), diagonal 128x128 sub-block
                 masked by a host 0/1 triangle tile (DVE bf16 mul).
  y^T   [D, S]   accumulated per head: lhsT = [v | 1] chunk, rhs = att^T span;
                 row 64 gives softmax sums. Normalize: DVE reciprocal of the
                 sum row, gpsimd partition_broadcast, one DVE mul that also
                 evicts psum -> yt bf16.
  out^T [D, S]   = W_p^T @ y^T (lhsT = wp chunk, rhs = y^T), fp32, transposed
                 back on host.

Emission order interleaves QK projection (fc) with attention (ft = fc-1) so
ACT exp work overlaps projection PE work.
"""

import os

# The Bass kernel executes through the axon PJRT backend and needs the
# NeuronCores visible; a JAX_PLATFORMS=cpu pin (used for jax reference
# computation) would hide them.
if "axon" not in os.environ.get("JAX_PLATFORMS", "axon"):
    os.environ.pop("JAX_PLATFORMS", None)

import numpy as np
import ml_dtypes
from contextlib import ExitStack

import concourse.bass as bass
import concourse.mybir as mybir
import concourse.tile as tile
from concourse import bacc
from concourse.bass_utils import run_bass_kernel_spmd

B, S, D, H, HS = 8, 1024, 1024, 16, 64
P = 128
NCORES = 8
F32 = mybir.dt.float32
BF16 = mybir.dt.bfloat16
EXP = mybir.ActivationFunctionType.Exp
BFNP = ml_dtypes.bfloat16

# Swap the two 16-row halves of each 32-partition quadrant (rotate-half
# partner exchange for the quadrant-pair RoPE layout).
SHUF_MASK = [(i + 16) % 32 for i in range(32)]

_CACHE = {}


def _build_nc():
    nc = bacc.Bacc(
        "TRN2", target_bir_lowering=False, debug=False, num_devices=NCORES)
    xT_d = nc.dram_tensor("xT", [D, S], BF16, kind="ExternalInput")
    wq_d = nc.dram_tensor("wq", [D, D], BF16, kind="ExternalInput")
    wk_d = nc.dram_tensor("wk", [D, D], BF16, kind="ExternalInput")
    wv_d = nc.dram_tensor("wv", [D, D], BF16, kind="ExternalInput")
    wp_d = nc.dram_tensor("wp", [D, D], BF16, kind="ExternalInput")
    c1q_d = nc.dram_tensor("c1q", [P, S], BF16, kind="ExternalInput")
    c2q_d = nc.dram_tensor("c2q", [P, S], BF16, kind="ExternalInput")
    c1k_d = nc.dram_tensor("c1k", [P, S], BF16, kind="ExternalInput")
    c2k_d = nc.dram_tensor("c2k", [P, S], BF16, kind="ExternalInput")
    mask_d = nc.dram_tensor("mask", [P, P], BF16, kind="ExternalInput")
    onesH_d = nc.dram_tensor("onesH", [P, H], BF16, kind="ExternalInput")
    outT_d = nc.dram_tensor("outT", [D, S], F32, kind="ExternalOutput")

    def mm(out, lhsT, rhs, start, stop):
        nc.tensor.matmul(out, lhsT, rhs, start=start, stop=stop)

    with tile.TileContext(nc) as tc, ExitStack() as ctx:
        persist = ctx.enter_context(tc.tile_pool(name="persist", bufs=1))
        xt = [persist.tile([P, S], BF16, name=f"xt{i}", tag=f"xt{i}") for i in range(8)]
        qt = [persist.tile([P, S], BF16, name=f"qt{i}", tag=f"qt{i}") for i in range(8)]
        kt = [persist.tile([P, S], BF16, name=f"kt{i}", tag=f"kt{i}") for i in range(8)]
        vt = [persist.tile([P, H, HS + 1], BF16, name=f"vt{i}", tag=f"vt{i}")
              for i in range(8)]
        yt = [persist.tile([P, S], BF16, name=f"yt{i}", tag=f"yt{i}") for i in range(8)]
        wqt = [persist.tile([P, S], BF16, name=f"wqt{i}", tag=f"wqt{i}") for i in range(8)]
        wkt = [persist.tile([P, S], BF16, name=f"wkt{i}", tag=f"wkt{i}") for i in range(8)]
        wpt = [persist.tile([P, S], BF16, name=f"wpt{i}", tag=f"wpt{i}") for i in range(8)]
        c1q = persist.tile([P, S], BF16, name="c1q_t", tag="c1q_t")
        c2q = persist.tile([P, S], BF16, name="c2q_t", tag="c2q_t")
        c1k = persist.tile([P, S], BF16, name="c1k_t", tag="c1k_t")
        c2k = persist.tile([P, S], BF16, name="c2k_t", tag="c2k_t")
        maskt = persist.tile([P, P], BF16, name="maskt", tag="maskt")
        onesH = persist.tile([P, H], BF16, name="onesH_t", tag="onesH_t")

        # PSUM pools: 2 + 4 + 2 = 8 banks.
        pbc = ctx.enter_context(tc.tile_pool(name="pbc", bufs=2, space="PSUM"))
        pss = ctx.enter_context(tc.tile_pool(name="pss", bufs=4, space="PSUM"))
        psy = ctx.enter_context(tc.tile_pool(name="psy", bufs=2, space="PSUM"))
        ropep = ctx.enter_context(tc.tile_pool(name="ropep", bufs=6))
        smallp = ctx.enter_context(tc.tile_pool(name="smallp", bufs=6))
        outp = ctx.enter_context(tc.tile_pool(name="outp", bufs=3))

        # wvt is only needed for phase C — scope it so its 16KB/partition is
        # released to attp afterwards.
        with tc.tile_pool(name="wvp", bufs=1) as wvp:
            wvt = [wvp.tile([P, S], BF16, name=f"wvt{i}", tag=f"wvt{i}")
                   for i in range(8)]
            # DMA order: x and wv first (V phase starts first). Split into
            # column strips issued across several engine queues so the first
            # V fills have their data within a few us (a whole 256KB tile on
            # one DMA engine takes ~11us).
            for i in range(8):
                nc.sync.dma_start(xt[i][:, 0:512], xT_d[i * P:(i + 1) * P, 0:512])
            for i in range(8):
                q = nc.gpsimd if i % 2 == 0 else nc.scalar
                q.dma_start(wvt[i][:, 0:512], wv_d[i * P:(i + 1) * P, 0:512])
            for i in range(8):
                nc.sync.dma_start(xt[i][:, 512:1024],
                                  xT_d[i * P:(i + 1) * P, 512:1024])
            for i in range(8):
                q = nc.gpsimd if i % 2 == 0 else nc.scalar
                q.dma_start(wvt[i][:, 512:1024],
                            wv_d[i * P:(i + 1) * P, 512:1024])
            for t, d_ in ((c1q, c1q_d), (c2q, c2q_d), (c1k, c1k_d),
                          (c2k, c2k_d), (maskt, mask_d), (onesH, onesH_d)):
                nc.sync.dma_start(t[:], d_[:])
            for i in range(8):
                nc.sync.dma_start(wqt[i][:], wq_d[i * P:(i + 1) * P, :])
            for i in range(8):
                nc.sync.dma_start(wkt[i][:], wk_d[i * P:(i + 1) * P, :])
            for i in range(8):
                nc.sync.dma_start(wpt[i][:], wp_d[i * P:(i + 1) * P, :])

            # ---------------- Phase C: V = x @ W_v ----------------
            for sc in range(8):
                for f2 in range(2):
                    ps = pbc.tile([P, 512], F32, name="vps", tag="pbc")
                    for dc in range(8):
                        mm(ps[:], xt[dc][:, sc * P:(sc + 1) * P],
                           wvt[dc][:, f2 * 512:(f2 + 1) * 512], dc == 0, dc == 7)
                    nc.scalar.copy(
                        vt[sc][:, f2 * 8:(f2 + 1) * 8, 0:HS],
                        ps[:].rearrange("p (h e) -> p h e", e=HS))
                nc.gpsimd.tensor_copy(vt[sc][:, :, HS], onesH[:])

        attp = ctx.enter_context(tc.tile_pool(name="attp", bufs=25))

        # ---------------- Phase B: Q^T/K^T + RoPE (per fc, per fill) -------
        B_UNITS = [(wqt, qt, c1q, c2q, 0), (wqt, qt, c1q, c2q, 1),
                   (wkt, kt, c1k, c2k, 0), (wkt, kt, c1k, c2k, 1)]

        def emit_B_fill(fc, j):
            wt, dstt, c1, c2, s2 = B_UNITS[j]
            ps = pbc.tile([P, 512], F32, name="qkps", tag="pbc")
            for dc in range(8):
                mm(ps[:], wt[dc][:, fc * P:(fc + 1) * P],
                   xt[dc][:, s2 * 512:(s2 + 1) * 512], dc == 0, dc == 7)
            s0 = s2 * 512
            qraw = ropep.tile([P, 512], BF16, name="qraw", tag="rope")
            nc.scalar.copy(qraw[:], ps[:])
            swp = ropep.tile([P, 512], BF16, name="swp", tag="rope")
            nc.vector.stream_shuffle(swp[:], qraw[:], SHUF_MASK)
            dst = dstt[fc][:, s0:s0 + 512]
            nc.vector.tensor_mul(dst, qraw[:], c1[:, s0:s0 + 512])
            t = ropep.tile([P, 512], BF16, name="ropet", tag="rope")
            nc.vector.tensor_mul(t[:], swp[:], c2[:, s0:s0 + 512])
            nc.vector.tensor_add(dst, dst, t[:])

        def emit_B(fc):
            for j in range(4):
                emit_B_fill(fc, j)

        # ---------------- Phase D: scores + att@V + normalize -------------
        # Emitted as a weave: groups of 4 score blocks (exactly filling the 4
        # pss banks) alternate with filler PE work (B fills, att@V chains, E
        # fills) so the PE never idles while ACT chews the exps.
        SQ = [(0, 0), (0, 1), (0, 2), (0, 3),
              (1, 0), (1, 1), (1, 2), (1, 3), (1, 4), (1, 5), (1, 6), (1, 7)]

        def emit_S_group(ft, g, atts):
            for qc, kc in SQ[2 * g:2 * g + 2]:
                dq = kc * P - qc * 512
                qsub = max(0, dq)
                for hb in (0, 64):
                    pst = pss.tile([P, 512], F32, name="pss", tag="pss")
                    mm(pst[:, qsub:],
                       kt[ft][hb:hb + 64, kc * P:(kc + 1) * P],
                       qt[ft][hb:hb + 64, qc * 512 + qsub:(qc + 1) * 512],
                       True, True)
                    att = attp.tile([P, 512], BF16, name="att", tag="att")
                    nc.scalar.activation(att[:, qsub:], pst[:, qsub:], EXP)
                    if 0 <= dq < 512:
                        nc.vector.tensor_mul(
                            att[:, dq:dq + P], att[:, dq:dq + P], maskt[:])
                    atts[(qc, kc, hb)] = (att, qsub)

        def emit_attV(ft, qc, hb, atts):
            kmax = 4 if qc == 0 else 8
            h = 2 * ft + hb // 64
            pyt = psy.tile([HS + 1, 512], F32, name="psy", tag="psy")
            for kc in range(kmax):
                att, qsub = atts[(qc, kc, hb)]
                mm(pyt[:, qsub:], vt[kc][:, h, :], att[:, qsub:],
                   kc == 0, kc == kmax - 1)
            # reciprocal_approx_fast must not read PSUM (bitwise seed reads
            # garbage) — stage the sum row through SBUF.
            srow = smallp.tile([1, 512], F32, name="srow", tag="rsb")
            nc.vector.tensor_copy(srow[:], pyt[HS:HS + 1, :])
            r_sb = smallp.tile([1, 512], F32, name="rsb", tag="rsb")
            nc.vector.reciprocal_approx_fast(out=r_sb[:], in_=srow[:])
            rb = smallp.tile([64, 512], F32, name="rb", tag="rb")
            nc.gpsimd.partition_broadcast(rb[:], r_sb[:])
            nc.vector.tensor_mul(
                yt[ft][hb:hb + 64, qc * 512:(qc + 1) * 512],
                pyt[0:HS, :], rb[:])

        def emit_E_fill(n8, qs, evict_dve):
            psp = pbc.tile([P, 512], F32, name="psp", tag="pbc")
            for dc in range(8):
                mm(psp[:], wpt[dc][:, n8 * P:(n8 + 1) * P],
                   yt[dc][:, qs * 512:(qs + 1) * 512], dc == 0, dc == 7)
            ot = outp.tile([P, 512], F32, name="ot", tag="ot")
            if evict_dve:
                nc.vector.tensor_copy(ot[:], psp[:])
            else:
                nc.scalar.copy(ot[:], psp[:])
            nc.sync.dma_start(
                outT_d[n8 * P:(n8 + 1) * P, qs * 512:(qs + 1) * 512], ot[:])

        def emit_D(ft):
            atts = {}
            if ft <= 5:
                gt = ft + 2
                fillers = [lambda j=j: emit_B_fill(gt, j) for j in range(4)]
                fillers += [lambda: emit_attV(ft, 0, 0, atts),
                            lambda: emit_attV(ft, 0, 64, atts)]
                for g in range(6):
                    emit_S_group(ft, g, atts)
                    fillers[g]()
            elif ft == 6:
                for g in range(6):
                    emit_S_group(ft, g, atts)
                    if g == 2:
                        emit_attV(ft, 0, 0, atts)
                    elif g == 3:
                        emit_attV(ft, 0, 64, atts)
            else:  # ft == 7: E fills for the q-first-half keep the PE busy
                emit_S_group(ft, 0, atts)
                emit_S_group(ft, 1, atts)
                emit_attV(ft, 0, 0, atts)
                emit_attV(ft, 0, 64, atts)
                for g in range(2, 6):
                    emit_S_group(ft, g, atts)
                    emit_E_fill(g - 2, 0, True)
            emit_attV(ft, 1, 0, atts)
            emit_attV(ft, 1, 64, atts)

        emit_B(0)
        emit_B(1)
        for ft in range(8):
            emit_D(ft)

        # ---------------- Phase E: remaining out^T fills ----------------
        # (n8 = 0..3, qs = 0 were already emitted as D7 fillers.)
        for n8 in range(4, 8):
            emit_E_fill(n8, 0, n8 % 2 == 0)
        for n8 in range(8):
            emit_E_fill(n8, 1, n8 % 2 == 0)
    nc.compile()
    return nc


def _prep(inputs):
    w_qkv = np.asarray(inputs["w_qkv"], np.float32)
    w_proj = np.asarray(inputs["w_proj"], np.float32)
    wq, wk, wv = w_qkv[:, 0:D], w_qkv[:, D:2 * D], w_qkv[:, 2 * D:3 * D]

    # Quadrant-pair RoPE layout. Within each head's 64 columns, new column i:
    #   qd = i//32 (quadrant), r = i%32, comp = r//16 (x1/x2), fl = r%16
    #   frequency f = qd*16 + fl ; original column = 2f + comp
    i = np.arange(64)
    qd, r = i // 32, i % 32
    comp, fl = r // 16, r % 16
    f = qd * 16 + fl
    base = np.repeat(np.arange(H) * 64, 64)
    perm = base + np.tile(2 * f + comp, H)
    wq, wk = wq[:, perm], wk[:, perm]

    # Coefficient tables [128, S]: rows repeat the 64-row head pattern twice.
    theta = 10000.0
    inv_freq = 1.0 / (theta ** (np.arange(0, HS, 2, dtype=np.float64) / HS))
    pos = np.arange(S, dtype=np.float64)
    ang = np.outer(inv_freq[f], pos)  # [64, S]
    sign = np.where(comp == 1, 1.0, -1.0)[:, None]
    c1_64 = np.cos(ang)
    c2_64 = sign * np.sin(ang)
    c1 = np.concatenate([c1_64, c1_64], 0).astype(np.float32)
    c2 = np.concatenate([c2_64, c2_64], 0).astype(np.float32)
    scale = np.float32(1.0 / np.sqrt(HS))

    mask = np.triu(np.ones((P, P), np.float32))  # [k, q]: allow q >= k
    common = {
        "wq": np.ascontiguousarray(wq).astype(BFNP),
        "wk": np.ascontiguousarray(wk).astype(BFNP),
        "wv": np.ascontiguousarray(wv).astype(BFNP),
        "wp": np.ascontiguousarray(w_proj).astype(BFNP),
        "c1q": (c1 * scale).astype(BFNP), "c2q": (c2 * scale).astype(BFNP),
        "c1k": c1.astype(BFNP), "c2k": c2.astype(BFNP),
        "mask": mask.astype(BFNP),
        "onesH": np.ones((P, H), BFNP),
    }
    return common


LAST_RESULT = None


def kernel(**inputs):
    global LAST_RESULT
    if "nc" not in _CACHE:
        _CACHE["nc"] = _build_nc()
    nc = _CACHE["nc"]
    common = _prep(inputs)
    x = np.asarray(inputs["x"], np.float32)
    in_maps = [
        dict(common, xT=np.ascontiguousarray(x[b].T).astype(BFNP))
        for b in range(B)
    ]
    res = run_bass_kernel_spmd(nc, in_maps, list(range(NCORES)))
    LAST_RESULT = res
    out = np.stack(
        [np.asarray(res.results[i]["outT"]).T for i in range(B)], 0)
    return np.ascontiguousarray(out).astype(np.float32)
